# revision 13
# baseline (speedup 1.0000x reference)
"""Trainium2 Bass kernel for windowed attention with relative-position bias.

Problem (hardcoded): x [32, 256, 25, 25] f32, w_qkv [256, 768], rel_emb [2401, 8],
w_out [256, 256], rel_idx [625, 625] int32. 8 heads of dim 32, n = 625 tokens.

Sharding: data-parallel over batch; 4 batches per core on 8 NeuronCores; weights
and bias tables replicated. No collectives.

Key idea vs the simple implementation: es = exp(sim + bias) is produced by a
SINGLE custom DVE instruction (EXP2BITS_ANT) that computes bf16 *bit patterns*
of 2^x directly: u = sim128 + biasA2_table (both in 128*log2 units, the +16256
exponent-bias folded into the q.k matmul via a constant contraction row), then
a float round-to-128-grid trick extracts the mantissa fraction and a quadratic
correction yields the bf16 bits, cast to int16 on output. Error ~0.6% rms.
A second path uses the Scalar engine's exact exp (scale folds 1/A2) followed by
a bf16 multiply with an exp(bias) table; tiles are statically assigned to
engines (DVE custom / Scalar exact / GpSimd Schraudolph) for load balance.

Per-core dataflow (bf16 matmuls, f32 PSUM):
  qkT planes: 8 sparse planes [q_h|1|pad|q_h'|1|pad] / [k_h|16256|pad|...] so
              each head-pair sim matmul has a 33-deep contraction that also
              adds the bf16 exponent bias constant.
  sim^T = k_h^T q_h  per (pair, jt); 2 heads concurrent on PE row groups 0/64.
  es    = one fused op per (head, jt) tile    -> bf16-bits SBUF
  av^T  = [den|v]^T @ es, accumulated over jt in PSUM; parity-alternating
          lane layout so normalize muls and outT rows line up with av lanes.
  1/den via reciprocal_approx_fast; broadcast across partitions with a
          DRAM-hop DMA; outT = av_sb * rcp (bf16 2x mode); project with
          lane-permuted w_out -> HBM.
Batches are software-pipelined as in the baseline (hooks interleave the next
batch's qkv work into the current batch's attention rounds).
"""

import sys

if "/opt/trn_rl_repo" not in sys.path:
    sys.path.insert(0, "/opt/trn_rl_repo")

import numpy as np
import ml_dtypes

B, D, WS = 32, 256, 25
N = WS * WS            # 625
NP = 640               # padded row length
H, DH = 8, 32
NC = 8                 # cores
BL = B // NC           # 4 batches per core
SCALE = DH ** -0.5
JT = 5                 # j tiles of 125
JP = N // JT           # 125
CHUNKS = ((0, 512), (512, 113))   # i chunks (PSUM bank rule)

A2 = 128.0 / np.log(2.0)          # 128*log2(e)
RHO = 16256.0                     # exponent-bias const (127*128), bf16-exact
EXP_C0 = float(2.0 ** 30 - 64.0)  # round-to-128-grid magic
EXP_C1 = 0.022                    # linear poly coeff (calibrated)
EXP_C2 = 0.0018                   # quadratic poly coeff (calibrated)
EXP_BETA = -9.8                   # additive calibration (mean-unbiased), in table
SCH_BETA = -8.27                  # Schraudolph offset (mean-unbiased), gp path
#   gp stt computes (sim128 + (SCH_BETA - EXP_BETA)) + table -> int16

# head h -> (plane, row) block in outT / av lanes (parity layout)
#   pair p = h//2; even p: h%2==0 -> rows 32:64, h%2==1 -> rows 96:128
#              odd p:  h%2==0 -> rows 0:32,  h%2==1 -> rows 64:96
def _head_rows(h):
    p, s = h // 2, h % 2
    if p % 2 == 0:
        return (32, 64) if s == 0 else (96, 128)
    return (0, 32) if s == 0 else (64, 96)


def _head_plane(h):
    return h // 4


# es engine assignment per (pair, jt, s): 'd' = DVE custom, 's' = scalar exact.
# (gpsimd cannot read PSUM, so it gets SBUF-only work instead.)
ES_MAP = {}
for _p in range(4):
    for _jt in range(JT):
        ES_MAP[(_p, _jt, 0)] = "d"
        ES_MAP[(_p, _jt, 1)] = "s"
# scalar-path bias-mul engine per (pair, jt, s): 'd' or 'g'
MUL_MAP = {}
_i = 0
for _k, _v in ES_MAP.items():
    if _v == "s":
        MUL_MAP[_k] = "d" if _i % 10 < 3 else "g"
        _i += 1

# qk plane cast engine per plane index 0..7
CAST_MAP = ["s", "d", "s", "d", "s", "d", "s", "d"]

PROJ_DMA_FROM_PSUM = False

_cache = {}


def _register_exp2bits():
    import concourse.dve_ops as dvo
    from concourse.dve_spec import Spec, Src0, Src1, C0, C1, C2, lower
    from concourse.dve_uop import DveOpSpec

    if "EXP2BITS_ANT" in dvo._SUB_OPCODE_FOR_NAME:
        return dvo._BY_NAME_EXP2BITS

    u = Src0 + Src1
    r = (u + C0) - C0
    f = u - r
    body = u + (C2 * f + C1) * f

    f32 = np.float32

    def ref(in0, in1, s0, s1, imm2):
        uu = f32(f32(in0) + f32(in1))
        rr = f32(f32(uu + f32(s0)) - f32(s0))
        ff = f32(uu - rr)
        return f32(uu + f32(f32(f32(imm2) * ff) + f32(s1)) * ff)

    spec = Spec(body=body, reference=ref)
    row = max(dvo._SUB_OPCODE_FOR_NAME.values()) + 1
    assert row < 0x20
    name = "EXP2BITS_ANT"
    # compute the uops sha in-process so the pin always matches
    shas = {}
    for ver in ("v3", "v4"):
        s = DveOpSpec(name=name, opcode=row, uops=lower(spec, ver=ver),
                      rd1_en=True)
        shas[ver] = s.sha(ver)
    op = dvo.DveOp(name=name, spec=spec, subdim=False, uops_sha=shas)
    dvo._SUB_OPCODE_FOR_NAME[name] = row
    dvo.OPS.append(op)
    dvo.CUSTOM_DVE_SPECS[name] = spec
    dvo._BY_NAME_EXP2BITS = op
    return op


def _build():
    import concourse.bass as bass
    from concourse import bacc, mybir
    from concourse.tile import TileContext

    f32 = mybir.dt.float32
    bf16 = mybir.dt.bfloat16
    i16 = mybir.dt.int16
    ADD = mybir.AluOpType.add

    exp2op = _register_exp2bits()

    nc = bacc.Bacc()
    x_ext = nc.declare_dram_parameter("x", [BL, D, N], bf16, isOutput=False)
    wpl_ext = nc.declare_dram_parameter("wpl", [D, 8, 128], bf16, isOutput=False)
    wv_ext = nc.declare_dram_parameter("wv", [D, D], bf16, isOutput=False)
    wout_ext = nc.declare_dram_parameter("wout", [D, D], bf16, isOutput=False)
    # tables: [H, JT, JP, NP] bf16
    ba2_ext = nc.declare_dram_parameter("biasA2", [H, JT, JP, NP], bf16, isOutput=False)
    eb_ext = nc.declare_dram_parameter("ebias", [H, JT, JP, NP], bf16, isOutput=False)
    out_ext = nc.declare_dram_parameter("out", [BL, D, N], f32, isOutput=True)

    with TileContext(nc) as tc:
        with (
            tc.tile_pool(name="const", bufs=1) as const,
            tc.tile_pool(name="xp", bufs=2) as xp,
            tc.tile_pool(name="qk", bufs=2) as qkp,
            tc.tile_pool(name="vp", bufs=2) as vp,
            tc.tile_pool(name="es", bufs=5) as esp,
            tc.tile_pool(name="esr", bufs=2) as esrp,
            tc.tile_pool(name="dn", bufs=3) as dnp,
            tc.tile_pool(name="avs", bufs=3) as avsp,
            tc.tile_pool(name="rb", bufs=3) as rbp,
            tc.tile_pool(name="ot", bufs=2) as otp,
            tc.tile_pool(name="res", bufs=2) as resp,
            tc.tile_pool(name="drp", bufs=6, space="DRAM") as drp,
            tc.tile_pool(name="sim", bufs=2, space="PSUM") as simp,
            tc.tile_pool(name="avb", bufs=1, space="PSUM") as avp,
            tc.tile_pool(name="acc", bufs=1, space="PSUM") as accp,
        ):
            wpl_sb = const.tile([128, 2, 8, 128], bf16)
            nc.sync.dma_start(out=wpl_sb, in_=wpl_ext.rearrange("(k p) l c -> p k l c", p=128))
            wv_sb = const.tile([128, 2, D], bf16)
            nc.sync.dma_start(out=wv_sb, in_=wv_ext.rearrange("(k p) c -> p k c", p=128))
            wout_sb = const.tile([128, 2, D], bf16)
            nc.sync.dma_start(out=wout_sb, in_=wout_ext.rearrange("(k p) c -> p k c", p=128))
            actb = const.tile([128, 1], f32)
            nc.vector.memset(actb, -RHO / A2)
            ba2_sb = const.tile([JP, H, JT, NP], bf16)
            eb_sb = const.tile([JP, H, JT, NP], bf16)
            # load in pair-major use order so batch 0 can start early
            for p in range(4):
                for h in (2 * p, 2 * p + 1):
                    for jt in range(JT):
                        nc.sync.dma_start(out=ba2_sb[:, h, jt, :], in_=ba2_ext[h, jt])
                        nc.sync.dma_start(out=eb_sb[:, h, jt, :], in_=eb_ext[h, jt])

            def x_load(b):
                x_sb = xp.tile([128, 2, N], bf16, tag="x", name=f"x_sb_{b}")
                nc.sync.dma_start(out=x_sb, in_=x_ext[b].rearrange("(k p) n -> p k n", p=128))
                return x_sb

            def qkv_pieces(b, x_sb):
                qkT_sb = qkp.tile([128, 8, NP], bf16, tag="qkT", name=f"qkT_{b}")
                v_sb = vp.tile([JP, JT, H, 2 * DH], bf16, tag="v", name=f"v_sb_{b}")

                def do_plane(pl):
                    ps = accp.tile([128, NP], f32, tag="acc", name=f"ps_{b}_{pl}")
                    for kt in range(2):
                        for lo, sz in CHUNKS:
                            nc.tensor.matmul(
                                ps[:, lo:lo + sz],
                                wpl_sb[:, kt, pl, :],
                                x_sb[:, kt, lo:lo + sz],
                                start=(kt == 0), stop=(kt == 1))
                    eng = {"s": nc.scalar, "d": nc.vector, "g": nc.gpsimd}[CAST_MAP[pl]]
                    if CAST_MAP[pl] == "s":
                        nc.scalar.copy(qkT_sb[0:97, pl, :N], ps[0:97, :N])
                    else:
                        eng.tensor_copy(qkT_sb[0:97, pl, :N], ps[0:97, :N])
                    val = 1.0 if pl < 4 else RHO
                    nc.gpsimd.memset(qkT_sb[32:33, pl, :], val)
                    nc.gpsimd.memset(qkT_sb[96:97, pl, :], val)

                def do_v(nt):
                    psv = accp.tile([128, NP], f32, tag="acc", name=f"psv_{b}_{nt}")
                    for kt in range(2):
                        nc.tensor.matmul(
                            psv[0:JP, :D],
                            x_sb[:, kt, nt * JP:(nt + 1) * JP],
                            wv_sb[:, kt, :],
                            start=(kt == 0), stop=(kt == 1))
                    hv = psv[0:JP, :D].rearrange("p (h d) -> p h d", h=H)
                    for h in range(H):
                        off = DH if (h // 2) % 2 == 0 else 0
                        nc.vector.tensor_copy(v_sb[:, nt, h, off:off + DH], hv[:, h, :])

                def do_consts():
                    for h in range(H):
                        if (h // 2) % 2 == 0:   # even pair: [1 | 0*31 | v]
                            nc.gpsimd.memset(v_sb[:, :, h, 0:1], 1.0)
                            nc.gpsimd.memset(v_sb[:, :, h, 1:DH], 0.0)
                        else:                   # odd pair: [v | 1 | 0*31]
                            nc.gpsimd.memset(v_sb[:, :, h, DH:DH + 1], 1.0)
                            nc.gpsimd.memset(v_sb[:, :, h, DH + 1:2 * DH], 0.0)

                pieces = [lambda pl=pl: do_plane(pl) for pl in range(8)]
                pieces += [lambda nt=nt: do_v(nt) for nt in range(JT)]
                pieces.append(do_consts)
                return (b, qkT_sb, v_sb), pieces

            def attention_phase(ctx, hooks=None):
                b, qkT_sb, v_sb = ctx
                hooks = hooks or {}
                outT_sb = otp.tile([128, 2, NP], bf16, tag="outT", name=f"outT_{b}")

                av_box = [None]
                muls_pend = []

                def finish_pair(p, av):
                    d0 = 0 if p % 2 == 0 else 32
                    av_fl = av.rearrange("p a b -> p (a b)")
                    # reciprocal of the two denominator rows (lanes d0, d0+64)
                    rcp = dnp.tile([128, NP], f32, tag="rcp", name=f"rcp_{b}_{p}")
                    nc.vector.reciprocal_approx_fast(
                        rcp[0:d0 + 65, 0:N], av_fl[0:d0 + 65, 0:N])
                    # dump to DRAM, broadcast back to the v lanes
                    rcp_d = drp.tile([2, N], f32, tag="rcpd", name=f"rcpd_{b}_{p}")
                    nc.sync.dma_start(out=rcp_d, in_=rcp[d0:d0 + 65:64, 0:N])
                    rb = rbp.tile([128, NP], f32, tag="rb", name=f"rb_{b}_{p}")
                    v0 = 32 - d0
                    ap0 = bass.AP(tensor=rcp_d.tensor, offset=rcp_d.offset,
                                  ap=[[0, 32], [1, N]])
                    ap1 = bass.AP(tensor=rcp_d.tensor, offset=rcp_d.offset + N,
                                  ap=[[0, 32], [1, N]])
                    nc.sync.dma_start(out=rb[v0:v0 + 32, 0:N], in_=ap0)
                    nc.sync.dma_start(out=rb[v0 + 64:v0 + 96, 0:N], in_=ap1)
                    # raw av -> SBUF bf16 (frees the av psum bank)
                    av_sb = avsp.tile([128, NP], bf16, tag="avsb", name=f"avsb_{b}_{p}")
                    nc.scalar.copy(av_sb[:, 0:N], av_fl[:, 0:N])
                    muls_pend.append((p, av_sb, rb))

                def issue_muls():
                    p, av_sb, rb = muls_pend.pop(0)
                    mt = p // 2
                    for s in range(2):
                        h = 2 * p + s
                        r0, r1 = _head_rows(h)
                        nc.gpsimd.tensor_mul(
                            outT_sb[r0:r1, mt, 0:N],
                            av_sb[r0:r1, 0:N],
                            rb[r0:r1, 0:N])

                def issue_av(p, jt, es_pair):
                    if jt == 0:
                        av_box[0] = avp.tile([128, 2, 512], f32, tag="av",
                                             name=f"av_{b}_{p}")
                    av = av_box[0]
                    for ci, (lo, sz) in enumerate(CHUNKS):
                        for s in range(2):
                            nc.tensor.matmul(
                                av[64 * s:64 * s + 2 * DH, ci, 0:sz],
                                v_sb[0:JP, jt, 2 * p + s, :],
                                es_pair[s][0:JP, lo:lo + sz],
                                start=(jt == 0), stop=(jt == JT - 1),
                                tile_position=(0, 64 * s),
                                skip_group_check=True)
                    if jt == JT - 1:
                        finish_pair(p, av)

                rounds = [(p, jt) for p in range(4) for jt in range(JT)]
                pend = []
                for r, (p, jt) in enumerate(rounds):
                    if r % JT == 3 and muls_pend:
                        issue_muls()
                    sims = [simp.tile([JP, NP], f32, tag="sim", name=f"sim_{b}_{r}_{s}")
                            for s in range(2)]
                    for ci, (lo, sz) in enumerate(CHUNKS):
                        for s in range(2):
                            nc.tensor.matmul(
                                sims[s][:, lo:lo + sz],
                                qkT_sb[64 * s:64 * s + DH + 1, 4 + p, jt * JP:(jt + 1) * JP],
                                qkT_sb[64 * s:64 * s + DH + 1, p, lo:lo + sz],
                                start=True, stop=True, tile_position=(64 * s, 0))
                    es_pair = []
                    for s in range(2):
                        h = 2 * p + s
                        eng = ES_MAP[(p, jt, s)]
                        es = esp.tile([JP, NP], bf16, tag="es", name=f"es_{b}_{r}_{s}")
                        if eng == "d":
                            nc.vector._custom_dve(
                                exp2op,
                                out=es[:, :N].bitcast(i16),
                                in0=sims[s][:, :N],
                                in1=ba2_sb[0:JP, h, jt, :N],
                                s0=EXP_C0, s1=EXP_C1, imm2=EXP_C2)
                        elif eng == "g":
                            nc.gpsimd.scalar_tensor_tensor(
                                out=es[:, :N].bitcast(i16),
                                in0=sims[s][:, :N],
                                scalar=SCH_BETA - EXP_BETA,
                                in1=ba2_sb[0:JP, h, jt, :N],
                                op0=ADD, op1=ADD)
                        else:
                            esr = esrp.tile([JP, NP], bf16, tag="esr",
                                            name=f"esr_{b}_{r}_{s}")
                            nc.scalar.activation(
                                out=esr[:, :N], in_=sims[s][:, :N],
                                func=mybir.ActivationFunctionType.Exp,
                                bias=actb[0:JP], scale=1.0 / A2)
                            meng = nc.vector if MUL_MAP[(p, jt, s)] == "d" else nc.gpsimd
                            meng.tensor_mul(es[:, :N], esr[:, :N],
                                            eb_sb[0:JP, h, jt, :N])
                        es_pair.append(es)
                    pend.append((p, jt, es_pair))
                    if len(pend) > 1:
                        pp, pjt, pes = pend.pop(0)
                        issue_av(pp, pjt, pes)
                    for fcb in hooks.get(r, ()):
                        fcb()
                while pend:
                    pp, pjt, pes = pend.pop(0)
                    issue_av(pp, pjt, pes)

                def do_proj():
                    for ct in range(2):
                        psp = accp.tile([128, NP], f32, tag="acc", name=f"psp_{b}_{ct}")
                        for kt in range(2):
                            for lo, sz in CHUNKS:
                                nc.tensor.matmul(
                                    psp[:, lo:lo + sz],
                                    wout_sb[:, kt, ct * 128:(ct + 1) * 128],
                                    outT_sb[:, kt, lo:lo + sz],
                                    start=(kt == 0), stop=(kt == 1))
                        if PROJ_DMA_FROM_PSUM:
                            nc.sync.dma_start(
                                out=out_ext[b, ct * 128:(ct + 1) * 128, :],
                                in_=psp[:, :N])
                        else:
                            o_t = resp.tile([128, NP], f32, tag="ot", name=f"o_t_{b}_{ct}")
                            nc.scalar.copy(o_t[:, :N], psp[:, :N])
                            nc.sync.dma_start(out=out_ext[b, ct * 128:(ct + 1) * 128, :],
                                              in_=o_t[:, :N])

                return [issue_muls, do_proj]

            # software pipeline across batches (baseline structure)
            x0 = x_load(0)
            ctx, pieces = qkv_pieces(0, x0)
            for piece in pieces:
                piece()
            fin = None
            for b in range(1, BL + 1):
                hooks = {}
                if fin is not None:
                    for i, fcb in enumerate(fin):
                        hooks.setdefault(i, []).append(fcb)
                if b < BL:
                    box = {}

                    def mk_xload(bb=b, box=box):
                        box["x"] = x_load(bb)

                    def mk_qkv(bb=b, box=box):
                        box["ctx"], box["pieces"] = qkv_pieces(bb, box["x"])
                        box["pieces"][0]()

                    hooks.setdefault(1, []).append(mk_xload)
                    hooks.setdefault(3, []).append(mk_qkv)
                    for i in range(1, 14):
                        def run_piece(i=i, box=box):
                            box["pieces"][i]()
                        hooks.setdefault(3 + i, []).append(run_piece)
                fin = attention_phase(ctx, hooks)
                if b < BL:
                    ctx = box["ctx"]
            for fcb in fin:
                fcb()

    nc.compile()
    return nc


def _get_nc():
    if "nc" not in _cache:
        _cache["nc"] = _build()
    return _cache["nc"]


def make_in_maps(x, w_qkv, rel_emb, w_out, rel_idx):
    bf = ml_dtypes.bfloat16
    f32 = np.float32
    wq = np.asarray(w_qkv[:, :D], f32) * f32(SCALE * A2)
    wk = np.asarray(w_qkv[:, D:2 * D], f32)
    wv = np.asarray(w_qkv[:, 2 * D:], f32).astype(bf)

    wpl = np.zeros((D, 8, 128), f32)
    for p in range(4):
        wpl[:, p, 0:32] = wq[:, (2 * p) * DH:(2 * p + 1) * DH]
        wpl[:, p, 64:96] = wq[:, (2 * p + 1) * DH:(2 * p + 2) * DH]
        wpl[:, 4 + p, 0:32] = wk[:, (2 * p) * DH:(2 * p + 1) * DH]
        wpl[:, 4 + p, 64:96] = wk[:, (2 * p + 1) * DH:(2 * p + 2) * DH]
    wpl = wpl.astype(bf)

    # w_out rows permuted per outT-v2 layout: (plane, row-block) <- head
    wout_p = np.zeros((D, D), f32)
    w_out = np.asarray(w_out, f32)
    for h in range(H):
        r0, _ = _head_rows(h)
        pl = _head_plane(h)
        wout_p[pl * 128 + r0: pl * 128 + r0 + DH, :] = w_out[h * DH:(h + 1) * DH, :]
    wout_p = wout_p.astype(bf)

    bias = np.asarray(rel_emb, f32)[np.asarray(rel_idx)]     # [i, j, h]
    biasT = np.ascontiguousarray(bias.transpose(2, 1, 0))    # [h, j, i]
    ba2 = np.zeros((H, JT, JP, NP), f32)
    ba2[..., :N] = (biasT * f32(A2) + f32(EXP_BETA)).reshape(H, JT, JP, N)
    ba2 = ba2.astype(bf)
    eb = np.zeros((H, JT, JP, NP), f32)
    eb[..., :N] = np.exp(biasT).reshape(H, JT, JP, N)
    eb = eb.astype(bf)

    xf = np.asarray(x, f32).reshape(B, D, N).astype(bf)
    return [
        {"x": xf[c * BL:(c + 1) * BL], "wpl": wpl, "wv": wv, "wout": wout_p,
         "biasA2": ba2, "ebias": eb}
        for c in range(NC)
    ]


def kernel(x, w_qkv, rel_emb, w_out, rel_idx):
    from concourse.bass_utils import run_bass_kernel_spmd

    nc = _get_nc()
    in_maps = make_in_maps(x, w_qkv, rel_emb, w_out, rel_idx)
    res = run_bass_kernel_spmd(nc, in_maps, list(range(NC)))
    out = np.concatenate([res.results[c]["out"] for c in range(NC)], axis=0)
    return out.reshape(B, D, WS, WS).astype(np.float32)


# revision 21
# speedup vs baseline: 1.1442x; 1.1442x over previous
"""Trainium2 Bass kernel for windowed attention with relative-position bias.

Problem (hardcoded): x [32, 256, 25, 25] f32, w_qkv [256, 768], rel_emb [2401, 8],
w_out [256, 256], rel_idx [625, 625] int32. 8 heads of dim 32, n = 625 tokens.

Sharding: data-parallel over batch; 4 batches per core on 8 NeuronCores; weights
and bias tables replicated. No collectives.

Key idea vs the simple implementation: es = exp(sim + bias) is produced by a
SINGLE custom DVE instruction (EXP2BITS_ANT) that computes bf16 *bit patterns*
of 2^x directly: u = sim128 + biasA2_table (both in 128*log2 units, the +16256
exponent-bias folded into the q.k matmul via a constant contraction row), then
a float round-to-128-grid trick extracts the mantissa fraction and a quadratic
correction yields the bf16 bits, cast to int16 on output. Error ~0.6% rms.
A second path uses the Scalar engine's exact exp (scale folds 1/A2) followed by
a bf16 multiply with an exp(bias) table; tiles are statically assigned to
engines (DVE custom / Scalar exact / GpSimd Schraudolph) for load balance.

Per-core dataflow (bf16 matmuls, f32 PSUM):
  qkT planes: 8 sparse planes [q_h|1|pad|q_h'|1|pad] / [k_h|16256|pad|...] so
              each head-pair sim matmul has a 33-deep contraction that also
              adds the bf16 exponent bias constant.
  sim^T = k_h^T q_h  per (pair, jt); 2 heads concurrent on PE row groups 0/64.
  es    = one fused op per (head, jt) tile    -> bf16-bits SBUF
  av^T  = [den|v]^T @ es, accumulated over jt in PSUM; parity-alternating
          lane layout so normalize muls and outT rows line up with av lanes.
  1/den via reciprocal_approx_fast; broadcast across partitions with a
          DRAM-hop DMA; outT = av_sb * rcp (bf16 2x mode); project with
          lane-permuted w_out -> HBM.
Batches are software-pipelined as in the baseline (hooks interleave the next
batch's qkv work into the current batch's attention rounds).
"""

import sys

if "/opt/trn_rl_repo" not in sys.path:
    sys.path.insert(0, "/opt/trn_rl_repo")

import numpy as np
import ml_dtypes

B, D, WS = 32, 256, 25
N = WS * WS            # 625
NP = 640               # padded row length
H, DH = 8, 32
NC = 8                 # cores
BL = B // NC           # 4 batches per core
SCALE = DH ** -0.5
JT = 5                 # j tiles of 125
JP = N // JT           # 125
CHUNKS = ((0, 512), (512, 113))   # i chunks (PSUM bank rule)

A2 = 128.0 / np.log(2.0)          # 128*log2(e)
RHO = 16256.0                     # exponent-bias const (127*128), bf16-exact
EXP_C0 = float(2.0 ** 30 - 64.0)  # round-to-128-grid magic
EXP_C1 = 0.022                    # linear poly coeff (calibrated)
EXP_C2 = 0.0018                   # quadratic poly coeff (calibrated)
EXP_BETA = -9.8                   # additive calibration (mean-unbiased), in table
SCH_BETA = -8.27                  # Schraudolph offset (mean-unbiased), gp path
#   gp stt computes (sim128 + (SCH_BETA - EXP_BETA)) + table -> int16

# head h -> (plane, row) block in outT / av lanes (parity layout)
#   pair p = h//2; even p: h%2==0 -> rows 32:64, h%2==1 -> rows 96:128
#              odd p:  h%2==0 -> rows 0:32,  h%2==1 -> rows 64:96
def _head_rows(h):
    p, s = h // 2, h % 2
    if p % 2 == 0:
        return (32, 64) if s == 0 else (96, 128)
    return (0, 32) if s == 0 else (64, 96)


def _head_plane(h):
    return h // 4


# es engine assignment per (pair, jt, s): 'd' = DVE custom, 's' = scalar exact.
# (gpsimd cannot read PSUM, so it gets SBUF-only work instead.)
ES_MAP = {}
for _p in range(4):
    for _jt in range(JT):
        ES_MAP[(_p, _jt, 0)] = "d"
        ES_MAP[(_p, _jt, 1)] = "s"
# scalar-path bias-mul engine per (pair, jt, s): 'd' or 'g'
MUL_MAP = {}
_i = 0
for _k, _v in ES_MAP.items():
    if _v == "s":
        MUL_MAP[_k] = "d" if _i % 5 == 0 else "g"
        _i += 1

# qk plane cast engine per plane index 0..7
CAST_MAP = ["s", "d", "s", "d", "s", "d", "s", "d"]

PROJ_DMA_FROM_PSUM = False

_cache = {}


def _register_exp2bits():
    import concourse.dve_ops as dvo
    from concourse.dve_spec import Spec, Src0, Src1, C0, C1, C2, lower
    from concourse.dve_uop import DveOpSpec

    if "EXP2BITS_ANT" in dvo._SUB_OPCODE_FOR_NAME:
        return dvo._BY_NAME_EXP2BITS

    u = Src0 + Src1
    r = (u + C0) - C0
    f = u - r
    body = u + (C2 * f + C1) * f

    f32 = np.float32

    def ref(in0, in1, s0, s1, imm2):
        uu = f32(f32(in0) + f32(in1))
        rr = f32(f32(uu + f32(s0)) - f32(s0))
        ff = f32(uu - rr)
        return f32(uu + f32(f32(f32(imm2) * ff) + f32(s1)) * ff)

    spec = Spec(body=body, reference=ref)
    row = max(dvo._SUB_OPCODE_FOR_NAME.values()) + 1
    assert row < 0x20
    name = "EXP2BITS_ANT"
    # compute the uops sha in-process so the pin always matches
    shas = {}
    for ver in ("v3", "v4"):
        s = DveOpSpec(name=name, opcode=row, uops=lower(spec, ver=ver),
                      rd1_en=True)
        shas[ver] = s.sha(ver)
    op = dvo.DveOp(name=name, spec=spec, subdim=False, uops_sha=shas)
    dvo._SUB_OPCODE_FOR_NAME[name] = row
    dvo.OPS.append(op)
    dvo.CUSTOM_DVE_SPECS[name] = spec
    dvo._BY_NAME_EXP2BITS = op
    return op


def _build():
    import concourse.bass as bass
    from concourse import bacc, mybir
    from concourse.tile import TileContext

    f32 = mybir.dt.float32
    bf16 = mybir.dt.bfloat16
    i16 = mybir.dt.int16
    ADD = mybir.AluOpType.add

    exp2op = _register_exp2bits()

    nc = bacc.Bacc()
    x_ext = nc.declare_dram_parameter("x", [BL, D, N], bf16, isOutput=False)
    wpl_ext = nc.declare_dram_parameter("wpl", [D, 8, 128], bf16, isOutput=False)
    wv_ext = nc.declare_dram_parameter("wv", [D, D], bf16, isOutput=False)
    wout_ext = nc.declare_dram_parameter("wout", [D, D], bf16, isOutput=False)
    # bias table: [H, JT, JP, NP] bf16 (ebias is derived on-device)
    ba2_ext = nc.declare_dram_parameter("biasA2", [H, JT, JP, NP], bf16, isOutput=False)
    out_ext = nc.declare_dram_parameter("out", [BL, D, N], f32, isOutput=True)

    with TileContext(nc) as tc:
        with (
            tc.tile_pool(name="const", bufs=1) as const,
            tc.tile_pool(name="xp", bufs=2) as xp,
            tc.tile_pool(name="qk", bufs=2) as qkp,
            tc.tile_pool(name="vp", bufs=2) as vp,
            tc.tile_pool(name="es", bufs=8) as esp,
            tc.tile_pool(name="esr", bufs=2) as esrp,
            tc.tile_pool(name="dn", bufs=3) as dnp,
            tc.tile_pool(name="avs", bufs=3) as avsp,
            tc.tile_pool(name="rb", bufs=3) as rbp,
            tc.tile_pool(name="ot", bufs=2) as otp,
            tc.tile_pool(name="res", bufs=2) as resp,
            tc.tile_pool(name="drp", bufs=6, space="DRAM") as drp,
            tc.tile_pool(name="sim", bufs=2, space="PSUM") as simp,
            tc.tile_pool(name="avb", bufs=1, space="PSUM") as avp,
            tc.tile_pool(name="acc", bufs=1, space="PSUM") as accp,
        ):
            wpl_sb = const.tile([128, 2, 8, 128], bf16)
            nc.sync.dma_start(out=wpl_sb, in_=wpl_ext.rearrange("(k p) l c -> p k l c", p=128))
            wv_sb = const.tile([128, 2, D], bf16)
            nc.sync.dma_start(out=wv_sb, in_=wv_ext.rearrange("(k p) c -> p k c", p=128))
            wout_sb = const.tile([128, 2, D], bf16)
            nc.scalar.dma_start(out=wout_sb, in_=wout_ext.rearrange("(k p) c -> p k c", p=128))
            actb = const.tile([128, 1], f32)
            nc.vector.memset(actb, -RHO / A2)
            rhot = const.tile([JP, NP], f32)
            nc.vector.memset(rhot, RHO)
            ba2_sb = const.tile([JP, H, JT, NP], bf16)
            eb_sb = const.tile([JP, H, JT, NP], bf16)

            def load_tables():
                # pair-major use order, split across both HWDGE queues, so
                # batch 0 can start as soon as the first pair's tables land
                for p in range(4):
                    for h in (2 * p, 2 * p + 1):
                        q = nc.sync if h % 2 == 0 else nc.scalar
                        for jt in range(JT):
                            q.dma_start(out=ba2_sb[:, h, jt, :], in_=ba2_ext[h, jt])
                # derive ebias = 2^((ba2)/128 ...) on-device: exp2bits(RHO + ba2)
                for p in range(4):
                    for h in (2 * p, 2 * p + 1):
                        for jt in range(JT):
                            nc.vector._custom_dve(
                                exp2op,
                                out=eb_sb[:, h, jt, :N].bitcast(i16),
                                in0=rhot[:, :N],
                                in1=ba2_sb[0:JP, h, jt, :N],
                                s0=EXP_C0, s1=EXP_C1, imm2=EXP_C2)

            def x_load(b):
                x_sb = xp.tile([128, 2, N], bf16, tag="x", name=f"x_sb_{b}")
                nc.sync.dma_start(out=x_sb, in_=x_ext[b].rearrange("(k p) n -> p k n", p=128))
                return x_sb

            def qkv_pieces(b, x_sb):
                qkT_sb = qkp.tile([128, 8, NP], bf16, tag="qkT", name=f"qkT_{b}")
                v_sb = vp.tile([JP, JT, H, 2 * DH], bf16, tag="v", name=f"v_sb_{b}")

                def do_plane(pl):
                    ps = accp.tile([128, NP], f32, tag="acc", name=f"ps_{b}_{pl}")
                    for kt in range(2):
                        for lo, sz in CHUNKS:
                            nc.tensor.matmul(
                                ps[:, lo:lo + sz],
                                wpl_sb[:, kt, pl, :],
                                x_sb[:, kt, lo:lo + sz],
                                start=(kt == 0), stop=(kt == 1))
                    eng = {"s": nc.scalar, "d": nc.vector, "g": nc.gpsimd}[CAST_MAP[pl]]
                    if CAST_MAP[pl] == "s":
                        nc.scalar.copy(qkT_sb[0:97, pl, :N], ps[0:97, :N])
                    else:
                        eng.tensor_copy(qkT_sb[0:97, pl, :N], ps[0:97, :N])
                    val = 1.0 if pl < 4 else RHO
                    nc.gpsimd.memset(qkT_sb[32:33, pl, :], val)
                    nc.gpsimd.memset(qkT_sb[96:97, pl, :], val)

                def do_v(nt):
                    psv = accp.tile([128, NP], f32, tag="acc", name=f"psv_{b}_{nt}")
                    for kt in range(2):
                        nc.tensor.matmul(
                            psv[0:JP, :D],
                            x_sb[:, kt, nt * JP:(nt + 1) * JP],
                            wv_sb[:, kt, :],
                            start=(kt == 0), stop=(kt == 1))
                    hv = psv[0:JP, :D].rearrange("p (h d) -> p h d", h=H)
                    for h in range(H):
                        off = DH if (h // 2) % 2 == 0 else 0
                        nc.vector.tensor_copy(v_sb[:, nt, h, off:off + DH], hv[:, h, :])

                def do_consts():
                    for h in range(H):
                        if (h // 2) % 2 == 0:   # even pair: [1 | 0*31 | v]
                            nc.gpsimd.memset(v_sb[:, :, h, 0:1], 1.0)
                            nc.gpsimd.memset(v_sb[:, :, h, 1:DH], 0.0)
                        else:                   # odd pair: [v | 1 | 0*31]
                            nc.gpsimd.memset(v_sb[:, :, h, DH:DH + 1], 1.0)
                            nc.gpsimd.memset(v_sb[:, :, h, DH + 1:2 * DH], 0.0)

                pieces = [lambda pl=pl: do_plane(pl) for pl in range(8)]
                pieces += [lambda nt=nt: do_v(nt) for nt in range(JT)]
                pieces.append(do_consts)
                return (b, qkT_sb, v_sb), pieces

            def attention_phase(ctx, hooks=None):
                b, qkT_sb, v_sb = ctx
                hooks = hooks or {}
                outT_sb = otp.tile([128, 2, NP], bf16, tag="outT", name=f"outT_{b}")

                av_box = [None]
                muls_pend = []

                def finish_pair(p, av):
                    d0 = 0 if p % 2 == 0 else 32
                    av_fl = av.rearrange("p a b -> p (a b)")
                    # reciprocal of the two denominator rows (lanes d0, d0+64)
                    rcp = dnp.tile([128, NP], f32, tag="rcp", name=f"rcp_{b}_{p}")
                    nc.vector.reciprocal_approx_fast(
                        rcp[0:d0 + 65, 0:N], av_fl[0:d0 + 65, 0:N])
                    # dump to DRAM, broadcast back to the v lanes
                    rcp_d = drp.tile([2, N], f32, tag="rcpd", name=f"rcpd_{b}_{p}")
                    nc.sync.dma_start(out=rcp_d, in_=rcp[d0:d0 + 65:64, 0:N])
                    rb = rbp.tile([128, NP], f32, tag="rb", name=f"rb_{b}_{p}")
                    v0 = 32 - d0
                    ap0 = bass.AP(tensor=rcp_d.tensor, offset=rcp_d.offset,
                                  ap=[[0, 32], [1, N]])
                    ap1 = bass.AP(tensor=rcp_d.tensor, offset=rcp_d.offset + N,
                                  ap=[[0, 32], [1, N]])
                    nc.sync.dma_start(out=rb[v0:v0 + 32, 0:N], in_=ap0)
                    nc.sync.dma_start(out=rb[v0 + 64:v0 + 96, 0:N], in_=ap1)
                    # raw av -> SBUF bf16 (frees the av psum bank)
                    av_sb = avsp.tile([128, NP], bf16, tag="avsb", name=f"avsb_{b}_{p}")
                    nc.scalar.copy(av_sb[:, 0:N], av_fl[:, 0:N])
                    muls_pend.append((p, av_sb, rb))

                def issue_muls():
                    p, av_sb, rb = muls_pend.pop(0)
                    mt = p // 2
                    for s in range(2):
                        h = 2 * p + s
                        r0, r1 = _head_rows(h)
                        nc.gpsimd.tensor_mul(
                            outT_sb[r0:r1, mt, 0:N],
                            av_sb[r0:r1, 0:N],
                            rb[r0:r1, 0:N])

                def issue_av(p, jt, es_pair):
                    if jt == 0:
                        av_box[0] = avp.tile([128, 2, 512], f32, tag="av",
                                             name=f"av_{b}_{p}")
                    av = av_box[0]
                    for ci, (lo, sz) in enumerate(CHUNKS):
                        for s in range(2):
                            nc.tensor.matmul(
                                av[64 * s:64 * s + 2 * DH, ci, 0:sz],
                                v_sb[0:JP, jt, 2 * p + s, :],
                                es_pair[s][0:JP, lo:lo + sz],
                                start=(jt == 0), stop=(jt == JT - 1),
                                tile_position=(0, 64 * s),
                                skip_group_check=True)
                    if jt == JT - 1:
                        finish_pair(p, av)

                rounds = [(p, jt) for p in range(4) for jt in range(JT)]
                pend = []
                for r, (p, jt) in enumerate(rounds):
                    if r % JT == 3 and muls_pend:
                        issue_muls()
                    sims = [simp.tile([JP, NP], f32, tag="sim", name=f"sim_{b}_{r}_{s}")
                            for s in range(2)]
                    for ci, (lo, sz) in enumerate(CHUNKS):
                        for s in range(2):
                            nc.tensor.matmul(
                                sims[s][:, lo:lo + sz],
                                qkT_sb[64 * s:64 * s + DH + 1, 4 + p, jt * JP:(jt + 1) * JP],
                                qkT_sb[64 * s:64 * s + DH + 1, p, lo:lo + sz],
                                start=True, stop=True, tile_position=(64 * s, 0))
                    es_pair = []
                    for s in range(2):
                        h = 2 * p + s
                        eng = ES_MAP[(p, jt, s)]
                        es = esp.tile([JP, NP], bf16, tag="es", name=f"es_{b}_{r}_{s}")
                        if eng == "d":
                            nc.vector._custom_dve(
                                exp2op,
                                out=es[:, :N].bitcast(i16),
                                in0=sims[s][:, :N],
                                in1=ba2_sb[0:JP, h, jt, :N],
                                s0=EXP_C0, s1=EXP_C1, imm2=EXP_C2)
                        elif eng == "g":
                            nc.gpsimd.scalar_tensor_tensor(
                                out=es[:, :N].bitcast(i16),
                                in0=sims[s][:, :N],
                                scalar=SCH_BETA - EXP_BETA,
                                in1=ba2_sb[0:JP, h, jt, :N],
                                op0=ADD, op1=ADD)
                        else:
                            esr = esrp.tile([JP, NP], bf16, tag="esr",
                                            name=f"esr_{b}_{r}_{s}")
                            nc.scalar.activation(
                                out=esr[:, :N], in_=sims[s][:, :N],
                                func=mybir.ActivationFunctionType.Exp,
                                bias=actb[0:JP], scale=1.0 / A2)
                            meng = nc.vector if MUL_MAP[(p, jt, s)] == "d" else nc.gpsimd
                            meng.tensor_mul(es[:, :N], esr[:, :N],
                                            eb_sb[0:JP, h, jt, :N])
                        es_pair.append(es)
                    pend.append((p, jt, es_pair))
                    if len(pend) > 2:
                        pp, pjt, pes = pend.pop(0)
                        issue_av(pp, pjt, pes)
                    for fcb in hooks.get(r, ()):
                        fcb()
                while pend:
                    pp, pjt, pes = pend.pop(0)
                    issue_av(pp, pjt, pes)

                def do_proj():
                    for ct in range(2):
                        psp = accp.tile([128, NP], f32, tag="acc", name=f"psp_{b}_{ct}")
                        for kt in range(2):
                            for lo, sz in CHUNKS:
                                nc.tensor.matmul(
                                    psp[:, lo:lo + sz],
                                    wout_sb[:, kt, ct * 128:(ct + 1) * 128],
                                    outT_sb[:, kt, lo:lo + sz],
                                    start=(kt == 0), stop=(kt == 1))
                        if PROJ_DMA_FROM_PSUM:
                            nc.sync.dma_start(
                                out=out_ext[b, ct * 128:(ct + 1) * 128, :],
                                in_=psp[:, :N])
                        else:
                            o_t = resp.tile([128, NP], f32, tag="ot", name=f"o_t_{b}_{ct}")
                            nc.scalar.copy(o_t[:, :N], psp[:, :N])
                            nc.sync.dma_start(out=out_ext[b, ct * 128:(ct + 1) * 128, :],
                                              in_=o_t[:, :N])

                return [issue_muls, do_proj]

            # software pipeline across batches (baseline structure)
            x0 = x_load(0)
            load_tables()
            ctx, pieces = qkv_pieces(0, x0)
            for piece in pieces:
                piece()
            fin = None
            for b in range(1, BL + 1):
                hooks = {}
                if fin is not None:
                    for i, fcb in enumerate(fin):
                        hooks.setdefault(i, []).append(fcb)
                if b < BL:
                    box = {}

                    def mk_xload(bb=b, box=box):
                        box["x"] = x_load(bb)

                    def mk_qkv(bb=b, box=box):
                        box["ctx"], box["pieces"] = qkv_pieces(bb, box["x"])
                        box["pieces"][0]()

                    hooks.setdefault(1, []).append(mk_xload)
                    hooks.setdefault(3, []).append(mk_qkv)
                    for i in range(1, 14):
                        def run_piece(i=i, box=box):
                            box["pieces"][i]()
                        hooks.setdefault(3 + i, []).append(run_piece)
                fin = attention_phase(ctx, hooks)
                if b < BL:
                    ctx = box["ctx"]
            for fcb in fin:
                fcb()

    nc.compile()
    return nc


def _get_nc():
    if "nc" not in _cache:
        _cache["nc"] = _build()
    return _cache["nc"]


def make_in_maps(x, w_qkv, rel_emb, w_out, rel_idx):
    bf = ml_dtypes.bfloat16
    f32 = np.float32
    wq = np.asarray(w_qkv[:, :D], f32) * f32(SCALE * A2)
    wk = np.asarray(w_qkv[:, D:2 * D], f32)
    wv = np.asarray(w_qkv[:, 2 * D:], f32).astype(bf)

    wpl = np.zeros((D, 8, 128), f32)
    for p in range(4):
        wpl[:, p, 0:32] = wq[:, (2 * p) * DH:(2 * p + 1) * DH]
        wpl[:, p, 64:96] = wq[:, (2 * p + 1) * DH:(2 * p + 2) * DH]
        wpl[:, 4 + p, 0:32] = wk[:, (2 * p) * DH:(2 * p + 1) * DH]
        wpl[:, 4 + p, 64:96] = wk[:, (2 * p + 1) * DH:(2 * p + 2) * DH]
    wpl = wpl.astype(bf)

    # w_out rows permuted per outT-v2 layout: (plane, row-block) <- head
    wout_p = np.zeros((D, D), f32)
    w_out = np.asarray(w_out, f32)
    for h in range(H):
        r0, _ = _head_rows(h)
        pl = _head_plane(h)
        wout_p[pl * 128 + r0: pl * 128 + r0 + DH, :] = w_out[h * DH:(h + 1) * DH, :]
    wout_p = wout_p.astype(bf)

    bias = np.asarray(rel_emb, f32)[np.asarray(rel_idx)]     # [i, j, h]
    biasT = np.ascontiguousarray(bias.transpose(2, 1, 0))    # [h, j, i]
    ba2 = np.zeros((H, JT, JP, NP), f32)
    ba2[..., :N] = (biasT * f32(A2) + f32(EXP_BETA)).reshape(H, JT, JP, N)
    ba2 = ba2.astype(bf)

    xf = np.asarray(x, f32).reshape(B, D, N).astype(bf)
    return [
        {"x": xf[c * BL:(c + 1) * BL], "wpl": wpl, "wv": wv, "wout": wout_p,
         "biasA2": ba2}
        for c in range(NC)
    ]


def kernel(x, w_qkv, rel_emb, w_out, rel_idx):
    from concourse.bass_utils import run_bass_kernel_spmd

    nc = _get_nc()
    in_maps = make_in_maps(x, w_qkv, rel_emb, w_out, rel_idx)
    res = run_bass_kernel_spmd(nc, in_maps, list(range(NC)))
    out = np.concatenate([res.results[c]["out"] for c in range(NC)], axis=0)
    return out.reshape(B, D, WS, WS).astype(np.float32)


# revision 31
# speedup vs baseline: 1.1981x; 1.0471x over previous
"""Trainium2 Bass kernel for windowed attention with relative-position bias.

Problem (hardcoded): x [32, 256, 25, 25] f32, w_qkv [256, 768], rel_emb [2401, 8],
w_out [256, 256], rel_idx [625, 625] int32. 8 heads of dim 32, n = 625 tokens.

Sharding: data-parallel over batch; 4 batches per core on 8 NeuronCores; weights
and bias tables replicated. No collectives.

Key idea vs the simple implementation: es = exp(sim + bias) is produced by a
SINGLE custom DVE instruction (EXP2BITS_ANT) that computes bf16 *bit patterns*
of 2^x directly: u = sim128 + biasA2_table (both in 128*log2 units, the +16256
exponent-bias folded into the q.k matmul via a constant contraction row), then
a float round-to-128-grid trick extracts the mantissa fraction and a quadratic
correction yields the bf16 bits, cast to int16 on output. Error ~0.6% rms.
A second path uses the Scalar engine's exact exp (scale folds 1/A2) followed by
a bf16 multiply with an exp(bias) table; tiles are statically assigned to
engines (DVE custom / Scalar exact / GpSimd Schraudolph) for load balance.

Per-core dataflow (bf16 matmuls, f32 PSUM):
  qkT planes: 8 sparse planes [q_h|1|pad|q_h'|1|pad] / [k_h|16256|pad|...] so
              each head-pair sim matmul has a 33-deep contraction that also
              adds the bf16 exponent bias constant.
  sim^T = k_h^T q_h  per (pair, jt); 2 heads concurrent on PE row groups 0/64.
  es    = one fused op per (head, jt) tile    -> bf16-bits SBUF
  av^T  = [den|v]^T @ es, accumulated over jt in PSUM; parity-alternating
          lane layout so normalize muls and outT rows line up with av lanes.
  1/den via reciprocal_approx_fast; broadcast across partitions with a
          DRAM-hop DMA; outT = av_sb * rcp (bf16 2x mode); project with
          lane-permuted w_out -> HBM.
Batches are software-pipelined as in the baseline (hooks interleave the next
batch's qkv work into the current batch's attention rounds).
"""

import sys

if "/opt/trn_rl_repo" not in sys.path:
    sys.path.insert(0, "/opt/trn_rl_repo")

import numpy as np
import ml_dtypes

B, D, WS = 32, 256, 25
N = WS * WS            # 625
NP = 640               # padded row length
H, DH = 8, 32
NC = 8                 # cores
BL = B // NC           # 4 batches per core
SCALE = DH ** -0.5
JT = 5                 # j tiles of 125
JP = N // JT           # 125
CHUNKS = ((0, 512), (512, 113))   # i chunks (PSUM bank rule)

A2 = 128.0 / np.log(2.0)          # 128*log2(e)
RHO = 16256.0                     # exponent-bias const (127*128), bf16-exact
EXP_C0 = float(2.0 ** 30 - 64.0)  # round-to-128-grid magic
EXP_C1 = 0.022                    # linear poly coeff (calibrated)
EXP_C2 = 0.0018                   # quadratic poly coeff (calibrated)
EXP_BETA = -9.8                   # additive calibration (mean-unbiased), in table
SCH_BETA = -8.27                  # Schraudolph offset (mean-unbiased), gp path
#   gp stt computes (sim128 + (SCH_BETA - EXP_BETA)) + table -> int16

# head h -> (plane, row) block in outT / av lanes (parity layout)
#   pair p = h//2; even p: h%2==0 -> rows 32:64, h%2==1 -> rows 96:128
#              odd p:  h%2==0 -> rows 0:32,  h%2==1 -> rows 64:96
def _head_rows(h):
    p, s = h // 2, h % 2
    if p % 2 == 0:
        return (32, 64) if s == 0 else (96, 128)
    return (0, 32) if s == 0 else (64, 96)


def _head_plane(h):
    return h // 4


# es engine assignment per (pair, jt, s): 'd' = DVE custom, 's' = scalar exact.
# (gpsimd cannot read PSUM, so it gets SBUF-only work instead.)
ES_MAP = {}
for _p in range(4):
    for _jt in range(JT):
        ES_MAP[(_p, _jt, 0)] = "d"
        ES_MAP[(_p, _jt, 1)] = "s"
# scalar-path bias-mul engine per (pair, jt, s): 'd' or 'g' (all gp: latency
# hidden by the 2-round av lag; gp is the only engine with slack)
MUL_MAP = {}
for _k, _v in ES_MAP.items():
    if _v == "s":
        MUL_MAP[_k] = "g"

# qk plane cast engine per plane index 0..7
CAST_MAP = ["s", "d", "s", "d", "s", "d", "s", "d"]

PROJ_DMA_FROM_PSUM = False

_cache = {}


def _register_exp2bits():
    import concourse.dve_ops as dvo
    from concourse.dve_spec import Spec, Src0, Src1, C0, C1, C2, lower
    from concourse.dve_uop import DveOpSpec

    if "EXP2BITS_ANT" in dvo._SUB_OPCODE_FOR_NAME:
        return dvo._BY_NAME_EXP2BITS

    u = Src0 + Src1
    r = (u + C0) - C0
    f = u - r
    body = u + (C2 * f + C1) * f

    f32 = np.float32

    def ref(in0, in1, s0, s1, imm2):
        uu = f32(f32(in0) + f32(in1))
        rr = f32(f32(uu + f32(s0)) - f32(s0))
        ff = f32(uu - rr)
        return f32(uu + f32(f32(f32(imm2) * ff) + f32(s1)) * ff)

    spec = Spec(body=body, reference=ref)
    row = max(dvo._SUB_OPCODE_FOR_NAME.values()) + 1
    assert row < 0x20
    name = "EXP2BITS_ANT"
    # compute the uops sha in-process so the pin always matches
    shas = {}
    for ver in ("v3", "v4"):
        s = DveOpSpec(name=name, opcode=row, uops=lower(spec, ver=ver),
                      rd1_en=True)
        shas[ver] = s.sha(ver)
    op = dvo.DveOp(name=name, spec=spec, subdim=False, uops_sha=shas)
    dvo._SUB_OPCODE_FOR_NAME[name] = row
    dvo.OPS.append(op)
    dvo.CUSTOM_DVE_SPECS[name] = spec
    dvo._BY_NAME_EXP2BITS = op
    return op


def _build():
    import concourse.bass as bass
    from concourse import bacc, mybir
    from concourse.tile import TileContext

    f32 = mybir.dt.float32
    bf16 = mybir.dt.bfloat16
    i16 = mybir.dt.int16
    ADD = mybir.AluOpType.add

    exp2op = _register_exp2bits()

    nc = bacc.Bacc()
    qkrows_ext = nc.declare_dram_parameter("qkrows", [2, NP], bf16, isOutput=False)
    x_ext = nc.declare_dram_parameter("x", [BL, D, N], bf16, isOutput=False)
    wpl_ext = nc.declare_dram_parameter("wpl", [D, 8, 128], bf16, isOutput=False)
    wv_ext = nc.declare_dram_parameter("wv", [D, D], bf16, isOutput=False)
    wout_ext = nc.declare_dram_parameter("wout", [D, D], bf16, isOutput=False)
    # bias table: [H, JT, JP, NP] bf16 (ebias is derived on-device)
    ba2_ext = nc.declare_dram_parameter("biasA2", [H, JT, JP, NP], bf16, isOutput=False)
    out_ext = nc.declare_dram_parameter("out", [BL, D, N], f32, isOutput=True)

    with TileContext(nc) as tc:
        with (
            tc.tile_pool(name="const", bufs=1) as const,
            tc.tile_pool(name="xp", bufs=2) as xp,
            tc.tile_pool(name="qk", bufs=2) as qkp,
            tc.tile_pool(name="vp", bufs=2) as vp,
            tc.tile_pool(name="es", bufs=8) as esp,
            tc.tile_pool(name="esr", bufs=2) as esrp,
            tc.tile_pool(name="dn", bufs=3) as dnp,
            tc.tile_pool(name="avs", bufs=3) as avsp,
            tc.tile_pool(name="rb", bufs=3) as rbp,
            tc.tile_pool(name="ot", bufs=2) as otp,
            tc.tile_pool(name="res", bufs=2) as resp,
            tc.tile_pool(name="drp", bufs=6, space="DRAM") as drp,
            tc.tile_pool(name="sim", bufs=2, space="PSUM") as simp,
            tc.tile_pool(name="avb", bufs=1, space="PSUM") as avp,
            tc.tile_pool(name="acc", bufs=1, space="PSUM") as accp,
        ):
            wpl_sb = const.tile([128, 2, 8, 128], bf16)
            nc.sync.dma_start(out=wpl_sb, in_=wpl_ext.rearrange("(k p) l c -> p k l c", p=128))
            wv_sb = const.tile([128, 2, D], bf16)
            nc.sync.dma_start(out=wv_sb, in_=wv_ext.rearrange("(k p) c -> p k c", p=128))
            wout_sb = const.tile([128, 2, D], bf16)
            nc.scalar.dma_start(out=wout_sb, in_=wout_ext.rearrange("(k p) c -> p k c", p=128))
            actb = const.tile([128, 1], f32)
            nc.vector.memset(actb, -RHO / A2)
            rhot = const.tile([JP, NP], f32)
            nc.vector.memset(rhot, RHO)
            ba2_sb = const.tile([JP, H, JT, NP], bf16)
            eb_sb = const.tile([JP, H, JT, NP], bf16)

            def load_tables():
                # pair-major use order, split across both HWDGE queues, so
                # batch 0 can start as soon as the first pair's tables land
                for p in range(4):
                    for h in (2 * p, 2 * p + 1):
                        q = nc.sync if h % 2 == 0 else nc.scalar
                        for jt in range(JT):
                            q.dma_start(out=ba2_sb[:, h, jt, :], in_=ba2_ext[h, jt])

                # ebias = 2^(ba2/128 + 127) on-device, only for scalar-path
                # (odd) heads; deferred per pair so the DVE queue isn't blocked
                def mk_derive(h):
                    def run():
                        for jt in range(JT):
                            nc.vector._custom_dve(
                                exp2op,
                                out=eb_sb[:, h, jt, :N].bitcast(i16),
                                in0=rhot[:, :N],
                                in1=ba2_sb[0:JP, h, jt, :N],
                                s0=EXP_C0, s1=EXP_C1, imm2=EXP_C2)
                    return run
                return [mk_derive(2 * p + 1) for p in range(4)]

            def x_load(b):
                x_sb = xp.tile([128, 2, N], bf16, tag="x", name=f"x_sb_{b}")
                nc.sync.dma_start(out=x_sb, in_=x_ext[b].rearrange("(k p) n -> p k n", p=128))
                return x_sb

            def qkv_pieces(b, x_sb):
                qkT_sb = qkp.tile([128, 8, NP], bf16, tag="qkT", name=f"qkT_{b}")
                v_sb = vp.tile([JP, JT, H, 2 * DH], bf16, tag="v", name=f"v_sb_{b}")

                def do_plane(pl):
                    ps = accp.tile([128, NP], f32, tag="acc", name=f"ps_{b}_{pl}")
                    for kt in range(2):
                        for lo, sz in CHUNKS:
                            nc.tensor.matmul(
                                ps[:, lo:lo + sz],
                                wpl_sb[:, kt, pl, :],
                                x_sb[:, kt, lo:lo + sz],
                                start=(kt == 0), stop=(kt == 1))
                    if CAST_MAP[pl] == "s":
                        nc.scalar.copy(qkT_sb[0:97, pl, :N], ps[0:97, :N])
                    else:
                        nc.vector.tensor_copy(qkT_sb[0:97, pl, :N], ps[0:97, :N])
                    # ones (q planes) / rho (k planes) rows via tiny DMAs --
                    # the DMA engine is the only unit with slack here
                    row = 0 if pl < 4 else 1
                    nc.sync.dma_start(out=qkT_sb[32:33, pl, :],
                                      in_=qkrows_ext[row:row + 1, :])
                    nc.scalar.dma_start(out=qkT_sb[96:97, pl, :],
                                        in_=qkrows_ext[row:row + 1, :])

                def do_v(nt):
                    psv = accp.tile([128, NP], f32, tag="acc", name=f"psv_{b}_{nt}")
                    for kt in range(2):
                        nc.tensor.matmul(
                            psv[0:JP, :D],
                            x_sb[:, kt, nt * JP:(nt + 1) * JP],
                            wv_sb[:, kt, :],
                            start=(kt == 0), stop=(kt == 1))
                    hv = psv[0:JP, :D].rearrange("p (h d) -> p h d", h=H)
                    for h in range(H):
                        off = DH if (h // 2) % 2 == 0 else 0
                        nc.vector.tensor_copy(v_sb[:, nt, h, off:off + DH], hv[:, h, :])

                def do_consts():
                    for h in range(H):
                        if (h // 2) % 2 == 0:   # even pair: [1 | 0*31 | v]
                            nc.gpsimd.memset(v_sb[:, :, h, 0:1], 1.0)
                            nc.gpsimd.memset(v_sb[:, :, h, 1:DH], 0.0)
                        else:                   # odd pair: [v | 1 | 0*31]
                            nc.gpsimd.memset(v_sb[:, :, h, DH:DH + 1], 1.0)
                            nc.gpsimd.memset(v_sb[:, :, h, DH + 1:2 * DH], 0.0)

                pieces = [lambda pl=pl: do_plane(pl) for pl in range(8)]
                pieces += [lambda nt=nt: do_v(nt) for nt in range(JT)]
                if b < 2:   # v buffers keep their const lanes across batches
                    pieces.append(do_consts)
                else:
                    pieces.append(lambda: None)
                return (b, qkT_sb, v_sb), pieces

            def attention_phase(ctx, hooks=None):
                b, qkT_sb, v_sb = ctx
                hooks = hooks or {}
                outT_sb = otp.tile([128, 2, NP], bf16, tag="outT", name=f"outT_{b}")

                av_box = [None]
                muls_pend = []

                def finish_pair(p, av):
                    d0 = 0 if p % 2 == 0 else 32
                    av_fl = av.rearrange("p a b -> p (a b)")
                    # reciprocal of the two denominator rows (lanes d0, d0+64)
                    rcp = dnp.tile([128, NP], f32, tag="rcp", name=f"rcp_{b}_{p}")
                    nc.vector.reciprocal_approx_fast(
                        rcp[0:d0 + 65, 0:N], av_fl[0:d0 + 65, 0:N])
                    rcb = dnp.tile([128, NP], bf16, tag="rcb", name=f"rcb_{b}_{p}")
                    nc.scalar.copy(rcb[0:d0 + 65, 0:N], rcp[0:d0 + 65, 0:N])
                    # dump to DRAM, broadcast back to the v lanes
                    rcp_d = drp.tile([2, N], bf16, tag="rcpd", name=f"rcpd_{b}_{p}")
                    nc.sync.dma_start(out=rcp_d, in_=rcb[d0:d0 + 65:64, 0:N])
                    rb = rbp.tile([128, NP], bf16, tag="rb", name=f"rb_{b}_{p}")
                    v0 = 32 - d0
                    ap0 = bass.AP(tensor=rcp_d.tensor, offset=rcp_d.offset,
                                  ap=[[0, 32], [1, N]])
                    ap1 = bass.AP(tensor=rcp_d.tensor, offset=rcp_d.offset + N,
                                  ap=[[0, 32], [1, N]])
                    nc.sync.dma_start(out=rb[v0:v0 + 32, 0:N], in_=ap0)
                    nc.sync.dma_start(out=rb[v0 + 64:v0 + 96, 0:N], in_=ap1)
                    # raw av -> SBUF bf16 (frees the av psum bank)
                    av_sb = avsp.tile([128, NP], bf16, tag="avsb", name=f"avsb_{b}_{p}")
                    nc.scalar.copy(av_sb[:, 0:N], av_fl[:, 0:N])
                    muls_pend.append((p, av_sb, rb))

                def issue_muls():
                    p, av_sb, rb = muls_pend.pop(0)
                    mt = p // 2
                    for s in range(2):
                        h = 2 * p + s
                        r0, r1 = _head_rows(h)
                        nc.vector.tensor_mul(
                            outT_sb[r0:r1, mt, 0:N],
                            av_sb[r0:r1, 0:N],
                            rb[r0:r1, 0:N])

                def issue_av(p, jt, es_pair):
                    if jt == 0:
                        av_box[0] = avp.tile([128, 2, 512], f32, tag="av",
                                             name=f"av_{b}_{p}")
                    av = av_box[0]
                    for ci, (lo, sz) in enumerate(CHUNKS):
                        for s in range(2):
                            nc.tensor.matmul(
                                av[64 * s:64 * s + 2 * DH, ci, 0:sz],
                                v_sb[0:JP, jt, 2 * p + s, :],
                                es_pair[s][0:JP, lo:lo + sz],
                                start=(jt == 0), stop=(jt == JT - 1),
                                tile_position=(0, 64 * s),
                                skip_group_check=True)
                    if jt == JT - 1:
                        finish_pair(p, av)

                rounds = [(p, jt) for p in range(4) for jt in range(JT)]
                pend = []
                for r, (p, jt) in enumerate(rounds):
                    if r % JT == 3 and muls_pend:
                        issue_muls()
                    sims = [simp.tile([JP, NP], f32, tag="sim", name=f"sim_{b}_{r}_{s}")
                            for s in range(2)]
                    for ci, (lo, sz) in enumerate(CHUNKS):
                        for s in range(2):
                            nc.tensor.matmul(
                                sims[s][:, lo:lo + sz],
                                qkT_sb[64 * s:64 * s + DH + 1, 4 + p, jt * JP:(jt + 1) * JP],
                                qkT_sb[64 * s:64 * s + DH + 1, p, lo:lo + sz],
                                start=True, stop=True, tile_position=(64 * s, 0))
                    es_pair = []
                    for s in range(2):
                        h = 2 * p + s
                        eng = ES_MAP[(p, jt, s)]
                        es = esp.tile([JP, NP], bf16, tag="es", name=f"es_{b}_{r}_{s}")
                        if eng == "d":
                            nc.vector._custom_dve(
                                exp2op,
                                out=es[:, :N].bitcast(i16),
                                in0=sims[s][:, :N],
                                in1=ba2_sb[0:JP, h, jt, :N],
                                s0=EXP_C0, s1=EXP_C1, imm2=EXP_C2)
                        elif eng == "g":
                            nc.gpsimd.scalar_tensor_tensor(
                                out=es[:, :N].bitcast(i16),
                                in0=sims[s][:, :N],
                                scalar=SCH_BETA - EXP_BETA,
                                in1=ba2_sb[0:JP, h, jt, :N],
                                op0=ADD, op1=ADD)
                        else:
                            esr = esrp.tile([JP, NP], bf16, tag="esr",
                                            name=f"esr_{b}_{r}_{s}")
                            nc.scalar.activation(
                                out=esr[:, :N], in_=sims[s][:, :N],
                                func=mybir.ActivationFunctionType.Exp,
                                bias=actb[0:JP], scale=1.0 / A2)
                            meng = nc.vector if MUL_MAP[(p, jt, s)] == "d" else nc.gpsimd
                            meng.tensor_mul(es[:, :N], esr[:, :N],
                                            eb_sb[0:JP, h, jt, :N])
                        es_pair.append(es)
                    pend.append((p, jt, es_pair))
                    if len(pend) > 2:
                        pp, pjt, pes = pend.pop(0)
                        issue_av(pp, pjt, pes)
                    for fcb in hooks.get(r, ()):
                        fcb()
                while pend:
                    pp, pjt, pes = pend.pop(0)
                    issue_av(pp, pjt, pes)

                def do_proj():
                    for ct in range(2):
                        psp = accp.tile([128, NP], f32, tag="acc", name=f"psp_{b}_{ct}")
                        for kt in range(2):
                            for lo, sz in CHUNKS:
                                nc.tensor.matmul(
                                    psp[:, lo:lo + sz],
                                    wout_sb[:, kt, ct * 128:(ct + 1) * 128],
                                    outT_sb[:, kt, lo:lo + sz],
                                    start=(kt == 0), stop=(kt == 1))
                        if PROJ_DMA_FROM_PSUM:
                            nc.sync.dma_start(
                                out=out_ext[b, ct * 128:(ct + 1) * 128, :],
                                in_=psp[:, :N])
                        else:
                            o_t = resp.tile([128, NP], f32, tag="ot", name=f"o_t_{b}_{ct}")
                            nc.scalar.copy(o_t[:, :N], psp[:, :N])
                            nc.sync.dma_start(out=out_ext[b, ct * 128:(ct + 1) * 128, :],
                                              in_=o_t[:, :N])

                return [issue_muls, do_proj]

            # software pipeline across batches (baseline structure)
            x0 = x_load(0)
            derives = load_tables()
            derives[0]()
            ctx, pieces = qkv_pieces(0, x0)
            for piece in pieces:
                piece()
            fin = None
            for b in range(1, BL + 1):
                hooks = {}
                if b == 1:
                    for i, dcb in ((0, derives[1]), (4, derives[2]), (8, derives[3])):
                        hooks.setdefault(i, []).append(dcb)
                if fin is not None:
                    for i, fcb in enumerate(fin):
                        hooks.setdefault(i, []).append(fcb)
                if b < BL:
                    box = {}

                    def mk_xload(bb=b, box=box):
                        box["x"] = x_load(bb)

                    def mk_qkv(bb=b, box=box):
                        box["ctx"], box["pieces"] = qkv_pieces(bb, box["x"])
                        box["pieces"][0]()

                    hooks.setdefault(1, []).append(mk_xload)
                    hooks.setdefault(3, []).append(mk_qkv)
                    for i in range(1, 14):
                        def run_piece(i=i, box=box):
                            box["pieces"][i]()
                        hooks.setdefault(3 + i, []).append(run_piece)
                fin = attention_phase(ctx, hooks)
                if b < BL:
                    ctx = box["ctx"]
            for fcb in fin:
                fcb()

    nc.compile()
    return nc


def _get_nc():
    if "nc" not in _cache:
        _cache["nc"] = _build()
    return _cache["nc"]


def make_in_maps(x, w_qkv, rel_emb, w_out, rel_idx):
    bf = ml_dtypes.bfloat16
    f32 = np.float32
    wq = np.asarray(w_qkv[:, :D], f32) * f32(SCALE * A2)
    wk = np.asarray(w_qkv[:, D:2 * D], f32)
    wv = np.asarray(w_qkv[:, 2 * D:], f32).astype(bf)

    wpl = np.zeros((D, 8, 128), f32)
    for p in range(4):
        wpl[:, p, 0:32] = wq[:, (2 * p) * DH:(2 * p + 1) * DH]
        wpl[:, p, 64:96] = wq[:, (2 * p + 1) * DH:(2 * p + 2) * DH]
        wpl[:, 4 + p, 0:32] = wk[:, (2 * p) * DH:(2 * p + 1) * DH]
        wpl[:, 4 + p, 64:96] = wk[:, (2 * p + 1) * DH:(2 * p + 2) * DH]
    wpl = wpl.astype(bf)

    # w_out rows permuted per outT-v2 layout: (plane, row-block) <- head
    wout_p = np.zeros((D, D), f32)
    w_out = np.asarray(w_out, f32)
    for h in range(H):
        r0, _ = _head_rows(h)
        pl = _head_plane(h)
        wout_p[pl * 128 + r0: pl * 128 + r0 + DH, :] = w_out[h * DH:(h + 1) * DH, :]
    wout_p = wout_p.astype(bf)

    bias = np.asarray(rel_emb, f32)[np.asarray(rel_idx)]     # [i, j, h]
    biasT = np.ascontiguousarray(bias.transpose(2, 1, 0))    # [h, j, i]
    ba2 = np.zeros((H, JT, JP, NP), f32)
    ba2[..., :N] = (biasT * f32(A2) + f32(EXP_BETA)).reshape(H, JT, JP, N)
    ba2 = ba2.astype(bf)

    qkrows = np.zeros((2, NP), f32)
    qkrows[0] = 1.0
    qkrows[1] = RHO
    qkrows = qkrows.astype(bf)

    xf = np.asarray(x, f32).reshape(B, D, N).astype(bf)
    return [
        {"x": xf[c * BL:(c + 1) * BL], "wpl": wpl, "wv": wv, "wout": wout_p,
         "biasA2": ba2, "qkrows": qkrows}
        for c in range(NC)
    ]


def kernel(x, w_qkv, rel_emb, w_out, rel_idx):
    from concourse.bass_utils import run_bass_kernel_spmd

    nc = _get_nc()
    in_maps = make_in_maps(x, w_qkv, rel_emb, w_out, rel_idx)
    res = run_bass_kernel_spmd(nc, in_maps, list(range(NC)))
    out = np.concatenate([res.results[c]["out"] for c in range(NC)], axis=0)
    return out.reshape(B, D, WS, WS).astype(np.float32)
